# revision 91
# baseline (speedup 1.0000x reference)
"""Dual-key additive attention (nn_Attention_58059367908033) on 8 trn2 NeuronCores.

Reference computation (per batch b, head h, n = 64*64 = 4096 positions, d = 128,
scale = d**-0.5):
    q  = Wq_h  @ fmap[b]          # [d, n]
    k1 = Wk1_h @ fmap[b]          # [d, n]
    v  = Wv_h  @ fmap[b]          # [d, n]
    k2 = Wk2_h @ x[b]             # [d, n]
    sim  = (scale * q)^T (k1+k2)  # [n, n]
    attn = softmax(sim, axis=-1)
    out[b, h*d:(h+1)*d] = (attn @ v^T)^T

Sharding: 8 cores = (b in 2) x (h in 2) x (key-half kh in 2).  Each core
computes unnormalized flash-attention partials over its 2048-key slice:
    U[d, q] = sum_k exp(scale*sim[k, q]) * vT[k, d]
    D[1, q] = sum_k exp(scale*sim[k, q])
and the host adds the two key-half partials and divides (exact softmax).

Measured: ~111us HW exec (baseline 117.6us), rel err 1.29e-2 (budget 2e-2).

Key optimizations over the bf16 baseline:
 - fp8 DoubleRow for the k2 projection (the only deep contraction): the
   16 per-pair channel-tile matmuls become 8 DoubleRow matmuls contracting
   2x128 channels (w2, x in fp8e4; Wk1/Wk2 pre-scaled by 16 on the host so
   fp8 stays in its normal range, the 1/16 folded into the exp scale).
   sim/PV/q stay bf16: their contraction is already 128, so fp8 DoubleRow
   cannot beat the merged-ksum bf16 path there.  x ships fp8: half the
   bytes.
 - DMA: all issued transfers are processed fair-share by the 16 SDMA
   engines and each dma_start costs ~0.65us of issue time on its
   sequencer, so only the first-exp critical set is issued up front
   (k1/q/v weights + fmap key columns on sync, x pair 0 chunked on the
   scalar ring, w2 + x pair 1 on gpsimd).  The ksum accumulation runs
   ct-chain first so it can start before fmap lands.  Later x pairs
   self-gate: the 2-buffer x pool makes their dma_start block at the
   issuing sequencer on the WAR semaphore of the previous occupant's ct
   matmuls.  Late fmap pieces ride gpsimd's FIFO behind tiny gate copies
   reading progressively later ksum columns.
 - Schedule: builds (ksum pumps, v halves, q chunks) are spread between
   sim tiles so the act engine's exp stream never starves; group-C PV
   units are woven into phase B so the post-last-exp tail stays short.
"""

from collections import deque

import ml_dtypes
import numpy as np

BF16_NP = ml_dtypes.bfloat16
F8_NP = ml_dtypes.float8_e4m3  # TRN float8e4 (IEEE-style e4m3, max 240)

import concourse.bass as bass
import concourse.mybir as mybir
import concourse.tile as tile
from concourse import bacc
from concourse.bass_utils import run_bass_kernel_spmd

HEADS = 2
D = 128          # dim head
C1 = 256         # fmap channels
C2 = 2048        # x channels
N = 4096         # spatial positions = queries; keys per core = 2048
KSL = 2048
KS = 16.0        # host pre-scale on Wk1/Wk2 (fp8 normal range)
SCALE = float(D) ** -0.5 / KS

F32 = mybir.dt.float32
BF16 = mybir.dt.bfloat16
FP8 = mybir.dt.float8e4
DR = mybir.MatmulPerfMode.DoubleRow

KC = 4           # key chunks per core (512 keys each)
QW = 512         # query-chunk width
QC = 8           # query chunks

_COMPILED = {}


def _build_program(warm1=8):
    nc = bacc.Bacc("TRN2", target_bir_lowering=False, debug=False, num_devices=8)

    # ---- DRAM parameters (per-core data, same program on all 8 cores) ----
    # w slots (bf16): wq=0:2, 16*wk1=2:4, wv=4:6  (each [128c, 128d])
    # w2 (fp8): [128ch, 8 tp, 2 sub, 128d]  16*wk2^T, channel tiles 2tp+sub
    # xs (fp8): [128ch, p*16+t, 256 keys]  pair-major
    d_w = nc.dram_tensor("w", [128, 6, 128], BF16, kind="ExternalInput").ap()
    d_w2 = nc.dram_tensor("w2", [128, 8, 2, 128], FP8, kind="ExternalInput").ap()
    d_fmap = nc.dram_tensor("fmap", [128, 2, N], BF16, kind="ExternalInput").ap()
    d_xs = nc.dram_tensor("xs", [128, 128, 256], FP8, kind="ExternalInput").ap()
    d_outU = nc.dram_tensor("outU", [128, N], BF16, kind="ExternalOutput").ap()
    d_den = nc.dram_tensor("denom", [1, N], BF16, kind="ExternalOutput").ap()

    with tile.TileContext(nc) as tc:
        with (
            tc.tile_pool(name="wts", bufs=1) as wts,
            tc.tile_pool(name="fm", bufs=1) as fm,
            tc.tile_pool(name="xs", bufs=2) as xsp,
            tc.tile_pool(name="big", bufs=1) as big,
            tc.tile_pool(name="etA", bufs=13) as etA,
            tc.tile_pool(name="etB", bufs=24) as etB,
            tc.tile_pool(name="etC", bufs=16) as etC,
            tc.tile_pool(name="stg", bufs=2) as stg,
            tc.tile_pool(name="ps_u", bufs=3, space="PSUM") as ps_u,
            tc.tile_pool(name="ps_s", bufs=2, space="PSUM") as ps_s,
            tc.tile_pool(name="ps_x", bufs=1, space="PSUM") as ps_x,
        ):
            w = wts.tile([128, 6, 128], BF16, tag="w")
            w2 = wts.tile([128, 8, 2, 128], FP8, tag="w2")
            fmap = fm.tile([128, 2, N], BF16, tag="fmap")
            x_tiles = [xsp.tile([128, 16, 256], FP8, tag="x", name=f"x{i}")
                       for i in range(2)]

            def load_xp(p, eng, t0=0, t1=16):
                eng.dma_start(x_tiles[p % 2][:, t0:t1],
                              d_xs[:, p * 16 + t0:p * 16 + t1, :])

            def load_fk(c0, c1, eng):
                eng.dma_start(fmap[:, :, c0:c1], d_fmap[:, :, c0:c1])

            ones = wts.tile([128, 1], BF16, tag="ones")
            nc.vector.memset(ones[:], 1.0)
            warm = wts.tile([128, 512], BF16, tag="warm")
            nc.vector.memset(warm[:], 0.0)

            # ---- first-wave DMAs (everything else is deferred).  Pieces
            # are ordered per ring by first consumer: the ksum ct chain
            # (w2 halves + x chunks) starts before fmap-dependent work.
            nc.sync.dma_start(w[:, 2:4], d_w[:, 2:4])       # k1 weights
            for c in range(4):                      # x pair 0, chunked
                load_xp(0, nc.scalar, 4 * c, 4 * c + 4)
            nc.gpsimd.dma_start(w2[:, 0:4], d_w2[:, 0:4])
            nc.gpsimd.dma_start(w2[:, 4:8], d_w2[:, 4:8])
            load_fk(0, 256, nc.sync)
            load_fk(256, 512, nc.sync)
            nc.sync.dma_start(w[:, 0:2], d_w[:, 0:2])       # q weights
            load_fk(512, 1024, nc.sync)
            nc.sync.dma_start(w[:, 4:6], d_w[:, 4:6])       # v weights
            for c in range(4):                      # x pair 1, chunked
                load_xp(1, nc.gpsimd, 4 * c, 4 * c + 4)
            load_fk(1024, 1536, nc.scalar)
            load_fk(1536, 2048, nc.gpsimd)

            # late fmap pieces ride gpsimd's FIFO behind tiny gate copies
            # that depend on progressively later ksum columns, so their
            # transfers cannot steal early DMA bandwidth
            gsc = wts.tile([128, 1], BF16, tag="gsc")

            def gate_on(col):
                nc.gpsimd.tensor_copy(gsc[:], ksum[:, col:col + 1])

            # ---- persistent SBUF tensors ----
            q_sb = big.tile([128, N], BF16, tag="q")
            ksum = big.tile([128, KSL], BF16, tag="ksum")
            vT = big.tile([128, 16, D], BF16, tag="vT")
            acc = big.tile([128, QC, 2, QW], BF16, tag="acc")
            den_stage = big.tile([1, N], BF16, tag="den")

            # ---- HAM warm-up: dummy matmuls keep the PE active during the
            # DMA-bound prologue so the clock gate opens early.
            def warmup(n):
                for _ in range(n):
                    wps = ps_x.tile([128, QW], F32, tag="x", name="pswarm")
                    nc.tensor.matmul(wps[:], warm[:, :128], warm[:],
                                     start=True, stop=True)

            warmup(warm1)

            # ---- vT tiles [k=128, d], built in halves of 2 key tiles so a
            # v build never starves the exp stream ----
            def build_v(g, h):
                psv = ps_x.tile([128, 2, D], F32, tag="x", name="psv")
                for i in range(2):
                    kk = g * 4 + h * 2 + i
                    ksl = slice(kk * 128, (kk + 1) * 128)
                    nc.tensor.matmul(psv[:, i, :], fmap[:, 0, ksl], w[:, 4, :],
                                     start=True, stop=False)
                    nc.tensor.matmul(psv[:, i, :], fmap[:, 1, ksl], w[:, 5, :],
                                     start=False, stop=True)
                nc.vector.tensor_copy(
                    vT[:, g * 4 + h * 2:g * 4 + h * 2 + 2, :], psv[:])

            # ---- q projection chunks ----
            def build_q(nch):
                psq = ps_s.tile([128, 2, QW], F32, tag="s", name="psq")
                sl = slice(nch * QW, (nch + 1) * QW)
                nc.tensor.matmul(psq[:, 0, :], w[:, 0, :], fmap[:, 0, sl],
                                 start=True, stop=False)
                nc.tensor.matmul(psq[:, 0, :], w[:, 1, :], fmap[:, 1, sl],
                                 start=False, stop=True)
                nc.vector.tensor_copy(q_sb[:, sl], psq[:, 0, :])

            # ---- ksum build per 256-key pair: 16*k1 (bf16) + 16*k2 (fp8
            # DoubleRow over channel-tile pairs) in one PSUM region, DVE
            # merge into ksum (bf16, carries the 16x). ----
            bq = deque()

            def queue_pair(p):
                kps = ps_x.tile([128, 512], F32, tag="x", name="kps")
                ra = kps[:, 0:256]
                sl = slice(p * 256, (p + 1) * 256)
                xt = x_tiles[p % 2]

                def k1a():
                    nc.tensor.matmul(ra, w[:, 2, :], fmap[:, 0, sl],
                                     start=False, stop=False)

                def k1b():
                    nc.tensor.matmul(ra, w[:, 3, :], fmap[:, 1, sl],
                                     start=False, stop=True)

                def ct(tp):
                    nc.tensor.matmul(ra, w2[:, tp], xt[:, 2 * tp:2 * tp + 2, :],
                                     start=(tp == 0), stop=False,
                                     perf_mode=DR)

                # ct chain first (its inputs land first), k1 appended last
                for tp in range(8):
                    bq.append(lambda tp=tp: ct(tp))
                bq.append(k1a)
                bq.append(k1b)
                bq.append(lambda: nc.vector.tensor_copy(ksum[:, sl], ra))

            def pump(n):
                for _ in range(min(n, len(bq))):
                    bq.popleft()()

            def flush():
                while bq:
                    bq.popleft()()

            queue_pair(0)
            flush()
            # deferred x issue: emitted after pair-0's ct matmuls exist so
            # the WAR semaphore paces the transfer behind them
            load_xp(2, nc.sync)
            build_q(0)
            build_q(1)

            # ---- attention machinery ----
            acc_first = [True] * QC
            pend = deque()          # deferred PE work (PV closures)

            def pop_pend():
                if pend:
                    pend.popleft()()

            def emit_sim_tile(kc, qc, half, pool):
                """sim matmuls + exp + denominator accumulate for key tiles
                (kc*4+2*half, +1) x query chunk qc.  Returns the exp tile."""
                sps = ps_s.tile([128, 2, QW], F32, tag="s", name="sps")
                qsl = slice(qc * QW, (qc + 1) * QW)
                for j in range(2):
                    kk = kc * 4 + 2 * half + j
                    nc.tensor.matmul(sps[:, j, :],
                                     ksum[:, kk * 128:(kk + 1) * 128],
                                     q_sb[:, qsl], start=True, stop=True)
                et = pool.tile([128, 2, QW], BF16, tag="et", name="et")
                nc.scalar.activation(et[:], sps[:],
                                     mybir.ActivationFunctionType.Exp,
                                     scale=SCALE)
                if acc_first[qc]:
                    nc.vector.tensor_copy(acc[:, qc], et[:])
                    acc_first[qc] = False
                else:
                    nc.vector.tensor_add(acc[:, qc], acc[:, qc], et[:])
                return et

            def make_pv(kc, qc, ets, U):
                def _pv():
                    for half in range(2):
                        for j in range(2):
                            kk = kc * 4 + 2 * half + j
                            nc.tensor.matmul(U[:], vT[:, kk, :],
                                             ets[half][:, j, :],
                                             start=(kc == 0 and kk == 0),
                                             stop=(kc == 3 and kk == 15))
                return _pv

            den_done = set()

            def emit_den(qc):
                qsl = slice(qc * QW, (qc + 1) * QW)
                dsum = stg.tile([128, QW], BF16, tag="dsum", name="dsum")
                nc.vector.tensor_add(dsum[:], acc[:, qc, 0, :], acc[:, qc, 1, :])
                dn = ps_x.tile([1, QW], F32, tag="x", name="dn")
                nc.tensor.matmul(dn[:], ones[:], dsum[:], start=True, stop=True)
                nc.vector.tensor_copy(den_stage[:, qsl], dn[:])
                den_done.add(qc)

            def drain(qc, U):
                if qc not in den_done:
                    emit_den(qc)
                qsl = slice(qc * QW, (qc + 1) * QW)
                u_st = stg.tile([128, QW], BF16, tag="u_st", name="u_st")
                nc.vector.tensor_copy(u_st[:], U[:])
                nc.sync.dma_start(d_outU[:, qsl], u_st[:])

            # ---- phase A: qc group {0,1,2} full attention; ksum builds,
            # v halves and q chunks spread between sim tiles; group-B exp
            # tiles stored for phase-B PV ----
            U_A = {qc: ps_u.tile([128, QW], F32, tag="u", name=f"ua{qc}")
                   for qc in (0, 1, 2)}
            etsB = {}
            ets = {}

            def T(p, qc):
                pool = etA if qc < 3 else etB
                e = emit_sim_tile(p // 2, qc, p % 2, pool)
                ets[(p, qc)] = e
                pump(4)
                pop_pend()
                if p % 2 == 1:
                    if qc < 3:
                        pend.append(make_pv(p // 2, qc,
                                            (ets[(p - 1, qc)], e), U_A[qc]))
                    else:
                        etsB[(p // 2, qc)] = (ets[(p - 1, qc)], e)

            sched = [
                ("QP", 1),
                ("T", 0, 0), ("T", 0, 1),
                ("F", 1), ("XP", 3), ("G", 256), ("FK", 2048, 2560),
                ("V", 0, 0), ("T", 1, 0), ("V", 0, 1),
                ("Q", 2), ("T", 1, 1),
                ("Q", 3), ("T", 0, 2), ("QP", 2),
                ("T", 1, 2), ("T", 0, 3), ("T", 1, 3),
                ("F", 2), ("XP", 4), ("G", 512), ("FK", 2560, 3072),
                ("Q", 4), ("T", 0, 4), ("QP", 3), ("T", 1, 4),
                ("T", 2, 0), ("V", 1, 0), ("T", 2, 1), ("V", 1, 1),
                ("T", 2, 2),
                ("Q", 5), ("T", 0, 5), ("T", 1, 5),
                ("F", 3), ("XP", 5),
                ("T", 3, 0), ("T", 3, 1),
                ("QP", 4), ("T", 3, 2),
                ("T", 2, 3), ("V", 2, 0), ("T", 2, 4), ("V", 2, 1),
                ("T", 2, 5),
                ("T", 3, 3), ("T", 3, 4), ("T", 3, 5),
                ("F", 4), ("XP", 6),
                ("G", 1024), ("FK", 3072, 3584), ("FK", 3584, 4096),
                ("T", 4, 0), ("T", 4, 1),
                ("QP", 5), ("T", 4, 2),
                ("T", 4, 3), ("V", 3, 0), ("T", 4, 4), ("V", 3, 1),
                ("T", 4, 5),
                ("F", 5), ("XP", 7), ("QP", 6),
                ("T", 5, 0), ("T", 5, 1), ("T", 5, 2),
                ("T", 5, 3), ("T", 5, 4), ("T", 5, 5),
                ("F", 6), ("QP", 7),
                ("T", 6, 0), ("T", 6, 1), ("T", 6, 2),
                ("T", 6, 3), ("T", 6, 4), ("T", 6, 5),
                ("F", 7),
                ("T", 7, 0), ("Q", 6), ("T", 7, 1), ("D", 0),
                ("T", 7, 2), ("D", 1), ("T", 7, 3), ("D", 2),
                ("T", 7, 4), ("T", 7, 5),
            ]
            etsC = {}

            def emitC(kc, qc, pump_n=0):
                ee0 = emit_sim_tile(kc, qc, 0, etC)
                ee1 = emit_sim_tile(kc, qc, 1, etC)
                etsC[(kc, qc)] = (ee0, ee1)

            for op in sched:
                if op[0] == "T":
                    T(op[1], op[2])
                elif op[0] == "QP":
                    queue_pair(op[1])
                elif op[0] == "F":
                    flush()
                elif op[0] == "XP":
                    load_xp(op[1], nc.sync)
                elif op[0] == "G":
                    gate_on(op[1])
                elif op[0] == "FK":
                    load_fk(op[1], op[2], nc.gpsimd)
                elif op[0] == "Q":
                    build_q(op[1])
                elif op[0] == "V":
                    build_v(op[1], op[2])
                elif op[0] == "D":
                    emit_den(op[1])
            while pend:
                pop_pend()

            # ---- phase B: PV for group B; group-A drains, q builds,
            # group-C sim/exp AND group-C PV units woven between B PV
            # units so the post-exp tail stays short ----
            preC = [(kc, qc) for qc in (6, 7) for kc in range(KC)]
            emitC(0, 6)
            ci = 1
            drain(0, U_A[0])
            # pre-allocate in drain order so the 3-slot pool rotation maps
            # each tile onto the slot whose previous occupant drains just
            # before the new tile's first PV write
            U_B = {qc: ps_u.tile([128, QW], F32, tag="u", name=f"ub{qc}")
                   for qc in (3, 4, 5)}
            U_C = {qc: ps_u.tile([128, QW], F32, tag="u", name=f"uc{qc}")
                   for qc in (6, 7)}
            cpv = deque()           # C-group PV units ready to run

            q7_built = [False]

            def weaveC(done_units):
                nonlocal ci
                # (kc, 7) sims may only be emitted after build_q(7) exists
                while ci < done_units + 2 and ci < (8 if q7_built[0] else 4):
                    emitC(*preC[ci])
                    ci += 1

            for qc in (3, 4, 5):
                if qc == 5:
                    emit_den(6)
                for kc in range(KC):
                    weaveC((qc - 3) * 4 + kc + 1)
                    make_pv(kc, qc, etsB[(kc, qc)], U_B[qc])()
                    if cpv:
                        cpv.popleft()()
                drain(qc, U_B[qc])
                if qc == 3:
                    drain(1, U_A[1])
                    build_q(7)
                    q7_built[0] = True
                    # qc6's C PVs can start: their U slot frees at drain(3)
                    for kc in range(KC):
                        cpv.append(make_pv(kc, 6, etsC[(kc, 6)], U_C[6]))
                elif qc == 4:
                    drain(2, U_A[2])
                    for kc in range(KC):
                        cpv.append(
                            lambda kc=kc: make_pv(kc, 7, etsC[(kc, 7)],
                                                  U_C[7])())

            # ---- phase C: leftover C PV units + drains ----
            emit_den(7)
            while cpv:
                cpv.popleft()()
            drain(6, U_C[6])
            drain(7, U_C[7])
            nc.sync.dma_start(d_den[:], den_stage[:])

    nc.compile()
    return nc


def _prep_inputs(fmap, x, Wqkv, Wk2):
    """Host-side slicing/transposition: per-core input dicts.
    Core c = b*4 + h*2 + kh.  fmap columns are rotated so the core's
    key half is always cols 0..2047 (outputs are un-rotated in _combine)."""
    fmap = np.ascontiguousarray(fmap, dtype=np.float32)
    x = np.ascontiguousarray(x, dtype=np.float32)
    Wqkv = np.ascontiguousarray(Wqkv, dtype=np.float32)
    Wk2 = np.ascontiguousarray(Wk2, dtype=np.float32)

    in_maps = []
    for c in range(8):
        b, h, kh = c // 4, (c // 2) % 2, c % 2
        fb = fmap[b].reshape(C1, N)
        xb = x[b].reshape(C2, N)
        # rotate fmap columns: key half first
        fb_r = np.roll(fb, -kh * KSL, axis=1)
        # qkv weights: [6, 128c_part, 128d] -> transpose to [128, 6, 128]
        w = np.empty((6, 128, 128), dtype=np.float32)
        wq = Wqkv[h * D:(h + 1) * D]              # [128, 256]
        wk1 = Wqkv[C1 + h * D:C1 + (h + 1) * D] * KS
        wv = Wqkv[2 * C1 + h * D:2 * C1 + (h + 1) * D]
        w[0:2] = wq.T.reshape(2, 128, D)
        w[2:4] = wk1.T.reshape(2, 128, D)
        w[4:6] = wv.T.reshape(2, 128, D)
        # 16*wk2^T per channel tile, fp8, channel-tile pairs in the DR slots
        wk2 = Wk2[h * D:(h + 1) * D] * KS         # [128, 2048]
        w2t = wk2.T.reshape(16, 128, D)           # [t, ch, d]
        w2 = np.ascontiguousarray(
            w2t.reshape(8, 2, 128, D).transpose(2, 0, 1, 3)).astype(F8_NP)
        # x key slice packed pair-major, fp8: [128 part, p*16+t, 256]
        xsl = xb[:, kh * KSL:(kh + 1) * KSL]      # [2048, 2048]
        xs = (xsl.reshape(16, 128, 8, 256)        # [t, part, pair, n]
                 .transpose(1, 2, 0, 3)           # [part, pair, t, n]
                 .reshape(128, 128, 256))
        in_maps.append({
            "w": np.ascontiguousarray(w.transpose(1, 0, 2)).astype(BF16_NP),
            "w2": w2,
            "fmap": np.ascontiguousarray(
                fb_r.reshape(2, 128, N).transpose(1, 0, 2)).astype(BF16_NP),
            "xs": xs.astype(F8_NP),
        })
    return in_maps


def _combine(results):
    """Host epilogue: un-rotate, add key-half partials, normalize."""
    out = np.empty((2, HEADS * D, 64, 64), dtype=np.float32)
    for b in range(2):
        for h in range(2):
            c0 = b * 4 + h * 2
            U0 = results[c0]["outU"].astype(np.float32)
            D0 = results[c0]["denom"]
            U1 = np.roll(results[c0 + 1]["outU"].astype(np.float32), KSL, axis=1)
            D1 = np.roll(results[c0 + 1]["denom"], KSL, axis=1)
            out[b, h * D:(h + 1) * D] = ((U0 + U1) / (D0 + D1)).reshape(D, 64, 64)
    return out


def run_on_device(in_maps, trace=False, **kw):
    if "nc" not in _COMPILED:
        _COMPILED["nc"] = _build_program()
    return run_bass_kernel_spmd(_COMPILED["nc"], in_maps, list(range(8)),
                                trace=trace, **kw)


def kernel(fmap, x, Wqkv, Wk2):
    in_maps = _prep_inputs(fmap, x, Wqkv, Wk2)
    res = run_on_device(in_maps)
    return _combine(res.results)
